# revision 5
# baseline (speedup 1.0000x reference)
# Trainium2 Bass kernel for nn_Attention_88313117540497.
#
# Reference computation (per batch b of 128):
#   v = x_b @ Wv                      (196, 384) @ (384, 512)
#   conv: each of the 512 channels' 14x14 image convolved with a 27x27
#         kernel qk at padding 13 -> same 14x14 output
#   y = conv_out @ Wo + bo            (196, 512) @ (512, 384)
#
# Key observations:
#  1. A 27x27 kernel on a 14x14 image with padding 13 covers every input
#     pixel for every output pixel, so the conv is exactly a dense linear
#     map over the 196 positions: out[p] = sum_u M[p, u] * img[u] with
#     M[(p,q),(u,v)] = qk[13+u-p, 13+v-q]. M is shared across all
#     batches and channels, so conv == matmul with a 196x196 matrix.
#  2. The whole module is then  y_b = M @ x_b @ Wv @ Wo + bo.  Folding
#     W = Wv @ Wo (384x384, computed once on device) removes the
#     INNER=512 dimension entirely: y_b = M @ (x_b @ W) + bo, which
#     halves the FLOPs.
#  3. Feeding x transposed (XT_b = x_b^T) makes both matmuls natural for
#     the PE (contraction dim on partitions for both operands, no
#     on-device transposes):
#        H_b = XT_b.T @ W      (lhsT = XT_b, rhs = W)   -> (196, 384)
#        Y_b = MT.T @ H_b      (lhsT = MT,   rhs = H_b) -> (196, 384)
#     with MT = M^T. All matmuls have free dim N = 384 >= 256, so
#     float32r runs at 1 cycle/row (4x faster than plain fp32).
#
# Sharding: data-parallel over batch, 16 batches per core, weights
# replicated. No collectives needed.

import numpy as np

import concourse.bass as bass
from concourse import bacc
import concourse.mybir as mybir
import concourse.tile as tile
from concourse.bass_utils import run_bass_kernel_spmd

N_CORES = 8
B = 128                 # total batch
BPC = B // N_CORES      # batches per core
DIM = 384
INNER = 512
NPOS = 196              # 14*14 positions
IMG = 14
KS = 27                 # conv kernel size

F32 = mybir.dt.float32
F32R = mybir.dt.float32r

# Token (196) split into PE-sized chunks along both contraction and
# output-partition dims.
TOK_CHUNKS = [(0, 128), (128, 68)]
# Feature dim (384) splits into 3 chunks of 128 for K, stays whole as N.
DCH = DIM // 128        # 3
ICH = INNER // 128      # 4

# Matmul input dtype: float32r (= tfloat32) is the PE's fast fp32 mode
# (full rate for free dim >= 256 vs 1/4 rate for plain float32). The BIR
# verifier requires every producer of an f32r matmul operand to write
# pre-rounded TF32, so DMA-fed operands are rounded to TF32 on the host
# (round-to-nearest on the mantissa) and declared float32r end-to-end,
# while on-chip producers (PSUM evictions) write float32r outputs.
MM_DT = F32R


def build_program():
    nc = bacc.Bacc("TRN2", debug=False)

    xt_d = nc.dram_tensor("xt", [BPC, DIM, NPOS], MM_DT, kind="ExternalInput")
    wvt_d = nc.dram_tensor("wvt", [INNER, DIM], MM_DT, kind="ExternalInput")
    wo_d = nc.dram_tensor("wo", [INNER, DIM], MM_DT, kind="ExternalInput")
    mt_d = nc.dram_tensor("mt", [NPOS, NPOS], MM_DT, kind="ExternalInput")
    bias_d = nc.dram_tensor("bias", [128, DIM], F32, kind="ExternalInput")
    y_d = nc.dram_tensor("y", [BPC, NPOS, DIM], F32, kind="ExternalOutput")

    with tile.TileContext(nc) as tc:
        with (
            tc.tile_pool(name="const", bufs=1) as const,
            tc.tile_pool(name="work", bufs=2) as work,
            tc.tile_pool(name="psum", bufs=2, space="PSUM") as psum,
        ):
            # ---- constants ----
            wvt_sb = const.tile([128, ICH * DIM], MM_DT)
            wo_sb = const.tile([128, ICH * DIM], MM_DT)
            for c in range(ICH):
                nc.sync.dma_start(
                    wvt_sb[:, c * DIM:(c + 1) * DIM],
                    wvt_d[c * 128:(c + 1) * 128, :],
                )
                nc.sync.dma_start(
                    wo_sb[:, c * DIM:(c + 1) * DIM],
                    wo_d[c * 128:(c + 1) * 128, :],
                )
            mt_sb = const.tile([128, 2 * NPOS], MM_DT)
            for uc, (u0, usz) in enumerate(TOK_CHUNKS):
                nc.sync.dma_start(
                    mt_sb[:usz, uc * NPOS:(uc + 1) * NPOS],
                    mt_d[u0:u0 + usz, :],
                )
            bias_sb = const.tile([128, DIM], F32)
            nc.sync.dma_start(bias_sb[:, :], bias_d[:, :])

            # ---- stage 0: fold W = Wv @ Wo  (W[d, e], d on partitions) ----
            w_sb = const.tile([128, DCH * DIM], MM_DT)
            fold_tags = ["h0", "h1", "y0"]
            for di in range(DCH):
                pw = psum.tile([128, DIM], F32, tag=fold_tags[di], name=f"pw{di}")
                for c in range(ICH):
                    nc.tensor.matmul(
                        pw[:, :],
                        lhsT=wvt_sb[:, c * DIM + di * 128:c * DIM + (di + 1) * 128],
                        rhs=wo_sb[:, c * DIM:(c + 1) * DIM],
                        start=(c == 0),
                        stop=(c == ICH - 1),
                    )
                nc.scalar.copy(w_sb[:, di * DIM:(di + 1) * DIM], pw[:, :])

            # ---- main loop over batches ----
            for b in range(BPC):
                # load XT_b (384, 196) as 3 partition-chunks side by side
                xt_t = work.tile([128, DCH * NPOS], MM_DT, tag="xt", bufs=3,
                                 name=f"xt{b}")
                nc.sync.dma_start(
                    xt_t[:, :].rearrange("p (c u) -> p c u", c=DCH),
                    xt_d[b].rearrange("(c p) u -> p c u", p=128),
                )

                # stage 1: H_b = XT_b.T @ W   (tokens on partitions, N=384)
                h_t = work.tile([128, 2 * DIM], MM_DT, tag="h", name=f"h{b}")
                for t, (u0, usz) in enumerate(TOK_CHUNKS):
                    ph = psum.tile([128, DIM], F32, tag=f"h{t}", name=f"ph{t}_{b}")
                    for c in range(DCH):
                        nc.tensor.matmul(
                            ph[:usz, :],
                            lhsT=xt_t[:, c * NPOS + u0:c * NPOS + u0 + usz],
                            rhs=w_sb[:, c * DIM:(c + 1) * DIM],
                            start=(c == 0),
                            stop=(c == DCH - 1),
                        )
                    nc.scalar.copy(h_t[:usz, t * DIM:(t + 1) * DIM], ph[:usz, :])

                # stage 2: Y_b = MT.T @ H_b + bias  (positions on partitions)
                y_t = work.tile([128, 2 * DIM], F32, tag="y", name=f"y{b}")
                for t2, (p0, psz) in enumerate(TOK_CHUNKS):
                    py = psum.tile([128, DIM], F32, tag=f"y{t2}", name=f"py{t2}_{b}")
                    for uc, (u0, usz) in enumerate(TOK_CHUNKS):
                        nc.tensor.matmul(
                            py[:psz, :],
                            lhsT=mt_sb[:usz, uc * NPOS + p0:uc * NPOS + p0 + psz],
                            rhs=h_t[:usz, uc * DIM:(uc + 1) * DIM],
                            start=(uc == 0),
                            stop=(uc == 1),
                        )
                    nc.vector.tensor_add(
                        y_t[:psz, t2 * DIM:(t2 + 1) * DIM],
                        py[:psz, :],
                        bias_sb[:psz, :],
                    )

                nc.sync.dma_start(y_d[b, 0:128, :], y_t[:, 0:DIM])
                nc.sync.dma_start(y_d[b, 128:NPOS, :], y_t[:68, DIM:2 * DIM])

    nc.compile()
    return nc


_PROGRAM = None


def _get_program():
    global _PROGRAM
    if _PROGRAM is None:
        _PROGRAM = build_program()
    return _PROGRAM


def _round_tf32(a):
    # round-to-nearest to the 10-bit TF32 mantissa (dtype-format conversion
    # for the float32r DRAM tensors; ties broken upward, which matches the
    # magnitude of HW round-to-nearest-even to within 1 ulp)
    b = (a.view(np.uint32) + np.uint32(0x1000)) & np.uint32(0xFFFFE000)
    return b.view(np.float32)


def _host_prep(x, Wv, qk, Wo, bo):
    x = np.asarray(x, dtype=np.float32)
    XT = _round_tf32(np.ascontiguousarray(x.transpose(0, 2, 1)))  # (B, 384, 196)
    WvT = _round_tf32(np.ascontiguousarray(np.asarray(Wv, np.float32).T))
    Wo = _round_tf32(np.ascontiguousarray(np.asarray(Wo, np.float32)))
    # MT[(u,v),(p,q)] = qk[13+u-p, 13+v-q]  (pure gather, no arithmetic)
    qk2 = np.asarray(qk, np.float32).reshape(KS, KS)
    idx = (KS // 2) + np.arange(IMG)[:, None] - np.arange(IMG)[None, :]
    MT = _round_tf32(np.ascontiguousarray(
        qk2[idx[:, None, :, None], idx[None, :, None, :]].reshape(NPOS, NPOS)
    ))
    bias = np.ascontiguousarray(
        np.broadcast_to(np.asarray(bo, np.float32), (128, DIM))
    )
    return XT, WvT, Wo, MT, bias


def _run(x, Wv, qk, Wo, bo, **spmd_kwargs):
    XT, WvT, Wo_, MT, bias = _host_prep(x, Wv, qk, Wo, bo)
    nc = _get_program()
    in_maps = [
        {
            "xt": XT[c * BPC:(c + 1) * BPC],
            "wvt": WvT,
            "wo": Wo_,
            "mt": MT,
            "bias": bias,
        }
        for c in range(N_CORES)
    ]
    res = run_bass_kernel_spmd(nc, in_maps, list(range(N_CORES)), **spmd_kwargs)
    y = np.concatenate([res.results[c]["y"] for c in range(N_CORES)], axis=0)
    return y, res


def kernel(x, Wv, qk, Wo, bo):
    y, _ = _run(x, Wv, qk, Wo, bo)
    return y


# revision 6
# speedup vs baseline: 1.0264x; 1.0264x over previous
# Trainium2 Bass kernel for nn_Attention_88313117540497.
#
# Reference computation (per batch b of 128):
#   v = x_b @ Wv                      (196, 384) @ (384, 512)
#   conv: each of the 512 channels' 14x14 image convolved with a 27x27
#         kernel qk at padding 13 -> same 14x14 output
#   y = conv_out @ Wo + bo            (196, 512) @ (512, 384)
#
# Key observations:
#  1. A 27x27 kernel on a 14x14 image with padding 13 covers every input
#     pixel for every output pixel, so the conv is exactly a dense linear
#     map over the 196 positions: out[p] = sum_u M[p, u] * img[u] with
#     M[(p,q),(u,v)] = qk[13+u-p, 13+v-q]. M is shared across all
#     batches and channels, so conv == matmul with a 196x196 matrix.
#  2. The whole module is then  y_b = M @ x_b @ Wv @ Wo + bo.  Folding
#     W = Wv @ Wo (384x384, computed once on device) removes the
#     INNER=512 dimension entirely: y_b = M @ (x_b @ W) + bo, which
#     halves the FLOPs.
#  3. Feeding x transposed (features major) makes both matmuls natural
#     for the PE (contraction dim on partitions for both operands, no
#     on-device transposes):
#        H_b = XT_b.T @ W      (lhsT = XT_b, rhs = W)   -> (196, 384)
#        Y_b = MT.T @ H_b      (lhsT = MT,   rhs = H_b) -> (196, 384)
#     with MT = M^T. All matmuls have free dim N = 384 >= 256, so
#     float32r runs at 1 cycle/row (4x faster than plain fp32).
#
# Sharding: data-parallel over batch, 16 batches per core, weights
# replicated. No collectives needed.
#
# DMA layout: x is pre-transposed on the host into a per-core
# (3, 128, 3136) block (feature-chunk, partition, token-stream) so each
# (chunk, partition) DRAM row is 12.5KB contiguous; X loads are issued
# per 4-batch group (3.1KB bursts). Outputs are written per 4-batch
# group as well to cut dma_start trigger overhead.

import numpy as np

import concourse.bass as bass
from concourse import bacc
import concourse.mybir as mybir
import concourse.tile as tile
from concourse.bass_utils import run_bass_kernel_spmd

N_CORES = 8
B = 128                 # total batch
BPC = B // N_CORES      # batches per core
DIM = 384
INNER = 512
NPOS = 196              # 14*14 positions
IMG = 14
KS = 27                 # conv kernel size

F32 = mybir.dt.float32
F32R = mybir.dt.float32r

# Token (196) split into PE-sized chunks along both contraction and
# output-partition dims.
TOK_CHUNKS = [(0, 128), (128, 68)]
DCH = DIM // 128        # 3 feature chunks (contraction of stage 1)
ICH = INNER // 128      # 4 inner chunks (contraction of the fold)
G = 4                   # batches per X-load / Y-store group
NG = BPC // G

# Matmul input dtype: float32r (= tfloat32) runs at full PE rate for
# free dim >= 256 (vs 1/4 rate for plain float32). The BIR verifier
# requires every producer of an f32r matmul operand to write pre-rounded
# TF32, so DMA-fed operands are rounded to TF32 on the host and declared
# float32r end-to-end, while on-chip producers (PSUM evictions) write
# float32r outputs.
MM_DT = F32R


def build_program():
    nc = bacc.Bacc("TRN2", debug=False)

    # x, feature-major: [feature chunk, partition (feature%128), token]
    xt_d = nc.dram_tensor("xt", [DCH, 128, BPC * NPOS], MM_DT,
                          kind="ExternalInput")
    wvt_d = nc.dram_tensor("wvt", [INNER, DIM], MM_DT, kind="ExternalInput")
    wo_d = nc.dram_tensor("wo", [INNER, DIM], MM_DT, kind="ExternalInput")
    mt_d = nc.dram_tensor("mt", [NPOS, NPOS], MM_DT, kind="ExternalInput")
    bias_d = nc.dram_tensor("bias", [128, DIM], F32, kind="ExternalInput")
    y_d = nc.dram_tensor("y", [BPC, NPOS, DIM], F32, kind="ExternalOutput")

    GT = G * NPOS        # tokens per group

    with tile.TileContext(nc) as tc:
        with (
            tc.tile_pool(name="const", bufs=1) as const,
            tc.tile_pool(name="work", bufs=2) as work,
            tc.tile_pool(name="psum", bufs=2, space="PSUM") as psum,
        ):
            # ---- constants (single strided DMA each) ----
            wvt_sb = const.tile([128, ICH * DIM], MM_DT)
            nc.sync.dma_start(
                wvt_sb[:, :].rearrange("p (c e) -> p c e", c=ICH),
                wvt_d.rearrange("(c p) e -> p c e", p=128),
            )
            wo_sb = const.tile([128, ICH * DIM], MM_DT)
            nc.sync.dma_start(
                wo_sb[:, :].rearrange("p (c e) -> p c e", c=ICH),
                wo_d.rearrange("(c p) e -> p c e", p=128),
            )
            mt_sb = const.tile([128, 2 * NPOS], MM_DT)
            for uc, (u0, usz) in enumerate(TOK_CHUNKS):
                nc.sync.dma_start(
                    mt_sb[:usz, uc * NPOS:(uc + 1) * NPOS],
                    mt_d[u0:u0 + usz, :],
                )
            bias_sb = const.tile([128, DIM], F32)
            nc.sync.dma_start(bias_sb[:, :], bias_d[:, :])

            # ---- stage 0: fold W = Wv @ Wo  (W[d, e], d on partitions) ----
            w_sb = const.tile([128, DCH * DIM], MM_DT)
            fold_tags = ["h0", "h1", "y0"]
            for di in range(DCH):
                pw = psum.tile([128, DIM], F32, tag=fold_tags[di], name=f"pw{di}")
                for c in range(ICH):
                    nc.tensor.matmul(
                        pw[:, :],
                        lhsT=wvt_sb[:, c * DIM + di * 128:c * DIM + (di + 1) * 128],
                        rhs=wo_sb[:, c * DIM:(c + 1) * DIM],
                        start=(c == 0),
                        stop=(c == ICH - 1),
                    )
                nc.scalar.copy(w_sb[:, di * DIM:(di + 1) * DIM], pw[:, :])

            # ---- main loop over batch groups ----
            for g in range(NG):
                # one DMA loads G batches of features:
                # tile free layout = [feature chunk, token-in-group]
                xt_t = work.tile([128, DCH * GT], MM_DT, tag="xt", bufs=3,
                                 name=f"xt{g}")
                nc.sync.dma_start(
                    xt_t[:, :].rearrange("p (c t) -> p c t", c=DCH),
                    xt_d[:, :, g * GT:(g + 1) * GT].rearrange("c p t -> p c t"),
                )

                # Y tile for the whole group: [batch-in-group, p-chunk, e]
                y_t = work.tile([128, G * 2 * DIM], F32, tag="y", name=f"y{g}")

                for bi in range(G):
                    tok0 = bi * NPOS   # within group

                    # stage 1: H_b = XT_b.T @ W  (tokens on partitions)
                    h_t = work.tile([128, 2 * DIM], MM_DT, tag="h", bufs=3,
                                    name=f"h{g}_{bi}")
                    for t, (u0, usz) in enumerate(TOK_CHUNKS):
                        ph = psum.tile([128, DIM], F32, tag=f"h{t}",
                                       name=f"ph{t}_{g}_{bi}")
                        for c in range(DCH):
                            o = c * GT + tok0 + u0
                            nc.tensor.matmul(
                                ph[:usz, :],
                                lhsT=xt_t[:, o:o + usz],
                                rhs=w_sb[:, c * DIM:(c + 1) * DIM],
                                start=(c == 0),
                                stop=(c == DCH - 1),
                            )
                        nc.scalar.copy(h_t[:usz, t * DIM:(t + 1) * DIM],
                                       ph[:usz, :])

                    # stage 2: Y_b = MT.T @ H_b + bias
                    for t2, (p0, psz) in enumerate(TOK_CHUNKS):
                        py = psum.tile([128, DIM], F32, tag=f"y{t2}",
                                       name=f"py{t2}_{g}_{bi}")
                        for uc, (u0, usz) in enumerate(TOK_CHUNKS):
                            nc.tensor.matmul(
                                py[:psz, :],
                                lhsT=mt_sb[:usz,
                                           uc * NPOS + p0:uc * NPOS + p0 + psz],
                                rhs=h_t[:usz, uc * DIM:(uc + 1) * DIM],
                                start=(uc == 0),
                                stop=(uc == 1),
                            )
                        nc.vector.tensor_add(
                            y_t[:psz, bi * 2 * DIM + t2 * DIM:
                                bi * 2 * DIM + (t2 + 1) * DIM],
                            py[:psz, :],
                            bias_sb[:psz, :],
                        )

                # two grouped output DMAs for the G batches
                y_view = y_t[:, :].rearrange("p (b k e) -> p b k e", b=G, k=2)
                nc.sync.dma_start(
                    y_d[g * G:(g + 1) * G, 0:128, :].rearrange("b p e -> p b e"),
                    y_view[:, :, 0, :],
                )
                nc.sync.dma_start(
                    y_d[g * G:(g + 1) * G, 128:NPOS, :].rearrange("b p e -> p b e"),
                    y_view[:68, :, 1, :],
                )

    nc.compile()
    return nc


_PROGRAM = None


def _get_program():
    global _PROGRAM
    if _PROGRAM is None:
        _PROGRAM = build_program()
    return _PROGRAM


def _round_tf32(a):
    # round-to-nearest to the 10-bit TF32 mantissa (dtype-format conversion
    # for the float32r DRAM tensors)
    b = (a.view(np.uint32) + np.uint32(0x1000)) & np.uint32(0xFFFFE000)
    return b.view(np.float32)


def _host_prep(x, Wv, qk, Wo, bo):
    x = np.asarray(x, dtype=np.float32)
    # per-core feature-major token stream: (cores, 3, 128, BPC*196)
    XTC = np.ascontiguousarray(
        x.reshape(N_CORES, BPC * NPOS, DIM).transpose(0, 2, 1)
    ).reshape(N_CORES, DCH, 128, BPC * NPOS)
    XTC = _round_tf32(XTC)
    WvT = _round_tf32(np.ascontiguousarray(np.asarray(Wv, np.float32).T))
    Wo = _round_tf32(np.ascontiguousarray(np.asarray(Wo, np.float32)))
    # MT[(u,v),(p,q)] = qk[13+u-p, 13+v-q]  (pure gather, no arithmetic)
    qk2 = np.asarray(qk, np.float32).reshape(KS, KS)
    idx = (KS // 2) + np.arange(IMG)[:, None] - np.arange(IMG)[None, :]
    MT = _round_tf32(np.ascontiguousarray(
        qk2[idx[:, None, :, None], idx[None, :, None, :]].reshape(NPOS, NPOS)
    ))
    bias = np.ascontiguousarray(
        np.broadcast_to(np.asarray(bo, np.float32), (128, DIM))
    )
    return XTC, WvT, Wo, MT, bias


def _run(x, Wv, qk, Wo, bo, **spmd_kwargs):
    XTC, WvT, Wo_, MT, bias = _host_prep(x, Wv, qk, Wo, bo)
    nc = _get_program()
    in_maps = [
        {"xt": XTC[c], "wvt": WvT, "wo": Wo_, "mt": MT, "bias": bias}
        for c in range(N_CORES)
    ]
    res = run_bass_kernel_spmd(nc, in_maps, list(range(N_CORES)), **spmd_kwargs)
    y = np.concatenate([res.results[c]["y"] for c in range(N_CORES)], axis=0)
    return y, res


def kernel(x, Wv, qk, Wo, bo):
    y, _ = _run(x, Wv, qk, Wo, bo)
    return y


# revision 8
# speedup vs baseline: 1.0781x; 1.0504x over previous
# Trainium2 Bass kernel for nn_Attention_88313117540497.
#
# Reference computation (per batch b of 128):
#   v = x_b @ Wv                      (196, 384) @ (384, 512)
#   conv: each of the 512 channels' 14x14 image convolved with a 27x27
#         kernel qk at padding 13 -> same 14x14 output
#   y = conv_out @ Wo + bo            (196, 512) @ (512, 384)
#
# Key observations:
#  1. A 27x27 kernel on a 14x14 image with padding 13 covers every input
#     pixel for every output pixel, so the conv is exactly a dense linear
#     map over the 196 positions: out[p] = sum_u M[p, u] * img[u] with
#     M[(p,q),(u,v)] = qk[13+u-p, 13+v-q]. M is shared across all
#     batches and channels, so conv == matmul with a 196x196 matrix.
#  2. The whole module is then  y_b = M @ x_b @ Wv @ Wo + bo.  Folding
#     W = Wv @ Wo (384x384, computed once on device) removes the
#     INNER=512 dimension entirely: y_b = M @ (x_b @ W) + bo, which
#     halves the FLOPs.
#  3. Feeding x transposed (features major) makes both matmuls natural
#     for the PE (contraction dim on partitions for both operands, no
#     on-device transposes):
#        H_b = XT_b.T @ W      (lhsT = XT_b, rhs = W)   -> (196, 384)
#        Y_b = MT.T @ H_b      (lhsT = MT,   rhs = H_b) -> (196, 384)
#     with MT = M^T. All matmuls have free dim N = 384 >= 256, so
#     float32r runs at 1 cycle/row (4x faster than plain fp32).
#
# Sharding: data-parallel over batch, 16 batches per core, weights
# replicated. No collectives needed.
#
# DMA layout: x is pre-transposed on the host into a per-core
# (3, 128, 3136) block (feature-chunk, partition, token-stream) so each
# (chunk, partition) DRAM row is 12.5KB contiguous; X loads are issued
# per 4-batch group (3.1KB bursts). Outputs are written per 4-batch
# group as well to cut dma_start trigger overhead.

import numpy as np

import concourse.bass as bass
from concourse import bacc
import concourse.mybir as mybir
import concourse.tile as tile
from concourse.bass_utils import run_bass_kernel_spmd

N_CORES = 8
B = 128                 # total batch
BPC = B // N_CORES      # batches per core
DIM = 384
INNER = 512
NPOS = 196              # 14*14 positions
IMG = 14
KS = 27                 # conv kernel size

F32 = mybir.dt.float32
F32R = mybir.dt.float32r

# Token (196) split into PE-sized chunks along both contraction and
# output-partition dims.
TOK_CHUNKS = [(0, 128), (128, 68)]
DCH = DIM // 128        # 3 feature chunks (contraction of stage 1)
ICH = INNER // 128      # 4 inner chunks (contraction of the fold)
G = 2                   # batches per X-load group
NG = BPC // G

# Matmul input dtype: float32r (= tfloat32) runs at full PE rate for
# free dim >= 256 (vs 1/4 rate for plain float32). The BIR verifier
# requires every producer of an f32r matmul operand to write pre-rounded
# TF32, so DMA-fed operands are rounded to TF32 on the host and declared
# float32r end-to-end, while on-chip producers (PSUM evictions) write
# float32r outputs.
MM_DT = F32R


def build_program():
    nc = bacc.Bacc("TRN2", debug=False)

    # x, feature-major: [feature chunk, partition (feature%128), token]
    xt_d = nc.dram_tensor("xt", [DCH, 128, BPC * NPOS], MM_DT,
                          kind="ExternalInput")
    wvt_d = nc.dram_tensor("wvt", [INNER, DIM], MM_DT, kind="ExternalInput")
    wo_d = nc.dram_tensor("wo", [INNER, DIM], MM_DT, kind="ExternalInput")
    mt_d = nc.dram_tensor("mt", [NPOS, NPOS], MM_DT, kind="ExternalInput")
    bias_d = nc.dram_tensor("bias", [128, DIM], F32, kind="ExternalInput")
    y_d = nc.dram_tensor("y", [BPC, NPOS, DIM], F32, kind="ExternalOutput")

    GT = G * NPOS        # tokens per group

    with tile.TileContext(nc) as tc:
        with (
            tc.tile_pool(name="const", bufs=1) as const,
            tc.tile_pool(name="work", bufs=2) as work,
            tc.tile_pool(name="psum", bufs=2, space="PSUM") as psum,
        ):
            # ---- constants (chunked DMAs spread over both HWDGE engines) ----
            dges = [nc.sync, nc.scalar]
            wvt_sb = const.tile([128, ICH * DIM], MM_DT)
            wo_sb = const.tile([128, ICH * DIM], MM_DT)
            for c in range(ICH):
                dges[c % 2].dma_start(
                    wvt_sb[:, c * DIM:(c + 1) * DIM],
                    wvt_d[c * 128:(c + 1) * 128, :],
                )
                dges[(c + 1) % 2].dma_start(
                    wo_sb[:, c * DIM:(c + 1) * DIM],
                    wo_d[c * 128:(c + 1) * 128, :],
                )
            mt_sb = const.tile([128, 2 * NPOS], MM_DT)
            for uc, (u0, usz) in enumerate(TOK_CHUNKS):
                dges[uc % 2].dma_start(
                    mt_sb[:usz, uc * NPOS:(uc + 1) * NPOS],
                    mt_d[u0:u0 + usz, :],
                )
            bias_sb = const.tile([128, DIM], F32)
            nc.sync.dma_start(bias_sb[:, :], bias_d[:, :])

            # ---- PE warm-up: ~4us of dummy matmuls gated on the first
            # weight chunk, so the tensor engine p-state is fully ramped
            # when the fold and stage-1 stream begin ----
            for wi in range(10):
                warm = psum.tile([128, DIM], F32, tag="y1", name=f"warm{wi}")
                nc.tensor.matmul(
                    warm[:, :],
                    lhsT=wvt_sb[:, 0:128],
                    rhs=wvt_sb[:, 0:DIM],
                    start=True,
                    stop=True,
                )

            # ---- stage 0: fold W = Wv @ Wo  (W[d, e], d on partitions) ----
            w_sb = const.tile([128, DCH * DIM], MM_DT)
            fold_tags = ["h0", "h1", "y0"]
            for di in range(DCH):
                pw = psum.tile([128, DIM], F32, tag=fold_tags[di], name=f"pw{di}")
                for c in range(ICH):
                    nc.tensor.matmul(
                        pw[:, :],
                        lhsT=wvt_sb[:, c * DIM + di * 128:c * DIM + (di + 1) * 128],
                        rhs=wo_sb[:, c * DIM:(c + 1) * DIM],
                        start=(c == 0),
                        stop=(c == ICH - 1),
                    )
                nc.scalar.copy(w_sb[:, di * DIM:(di + 1) * DIM], pw[:, :])

            # ---- main loop over batch groups ----
            for g in range(NG):
                # one DMA loads G batches of features:
                # tile free layout = [feature chunk, token-in-group]
                xt_t = work.tile([128, DCH * GT], MM_DT, tag="xt", bufs=3,
                                 name=f"xt{g}")
                dges[g % 2].dma_start(
                    xt_t[:, :].rearrange("p (c t) -> p c t", c=DCH),
                    xt_d[:, :, g * GT:(g + 1) * GT].rearrange("c p t -> p c t"),
                )

                # Y tile for the whole group: [batch-in-group, p-chunk, e]
                y_t = work.tile([128, G * 2 * DIM], F32, tag="y", name=f"y{g}")

                for bi in range(G):
                    tok0 = bi * NPOS   # within group

                    # stage 1: H_b = XT_b.T @ W  (tokens on partitions)
                    h_t = work.tile([128, 2 * DIM], MM_DT, tag="h", bufs=3,
                                    name=f"h{g}_{bi}")
                    for t, (u0, usz) in enumerate(TOK_CHUNKS):
                        ph = psum.tile([128, DIM], F32, tag=f"h{t}",
                                       name=f"ph{t}_{g}_{bi}")
                        for c in range(DCH):
                            o = c * GT + tok0 + u0
                            nc.tensor.matmul(
                                ph[:usz, :],
                                lhsT=xt_t[:, o:o + usz],
                                rhs=w_sb[:, c * DIM:(c + 1) * DIM],
                                start=(c == 0),
                                stop=(c == DCH - 1),
                            )
                        nc.scalar.copy(h_t[:usz, t * DIM:(t + 1) * DIM],
                                       ph[:usz, :])

                    # stage 2: Y_b = MT.T @ H_b + bias
                    for t2, (p0, psz) in enumerate(TOK_CHUNKS):
                        py = psum.tile([128, DIM], F32, tag=f"y{t2}",
                                       name=f"py{t2}_{g}_{bi}")
                        for uc, (u0, usz) in enumerate(TOK_CHUNKS):
                            nc.tensor.matmul(
                                py[:psz, :],
                                lhsT=mt_sb[:usz,
                                           uc * NPOS + p0:uc * NPOS + p0 + psz],
                                rhs=h_t[:usz, uc * DIM:(uc + 1) * DIM],
                                start=(uc == 0),
                                stop=(uc == 1),
                            )
                        nc.vector.tensor_add(
                            y_t[:psz, bi * 2 * DIM + t2 * DIM:
                                bi * 2 * DIM + (t2 + 1) * DIM],
                            py[:psz, :],
                            bias_sb[:psz, :],
                        )

                # two grouped output DMAs for the G batches
                y_view = y_t[:, :].rearrange("p (b k e) -> p b k e", b=G, k=2)
                dges[g % 2].dma_start(
                    y_d[g * G:(g + 1) * G, 0:128, :].rearrange("b p e -> p b e"),
                    y_view[:, :, 0, :],
                )
                dges[(g + 1) % 2].dma_start(
                    y_d[g * G:(g + 1) * G, 128:NPOS, :].rearrange("b p e -> p b e"),
                    y_view[:68, :, 1, :],
                )

    nc.compile()
    return nc


_PROGRAM = None


def _get_program():
    global _PROGRAM
    if _PROGRAM is None:
        _PROGRAM = build_program()
    return _PROGRAM


def _round_tf32(a):
    # round-to-nearest to the 10-bit TF32 mantissa (dtype-format conversion
    # for the float32r DRAM tensors)
    b = (a.view(np.uint32) + np.uint32(0x1000)) & np.uint32(0xFFFFE000)
    return b.view(np.float32)


def _host_prep(x, Wv, qk, Wo, bo):
    x = np.asarray(x, dtype=np.float32)
    # per-core feature-major token stream: (cores, 3, 128, BPC*196)
    XTC = np.ascontiguousarray(
        x.reshape(N_CORES, BPC * NPOS, DIM).transpose(0, 2, 1)
    ).reshape(N_CORES, DCH, 128, BPC * NPOS)
    XTC = _round_tf32(XTC)
    WvT = _round_tf32(np.ascontiguousarray(np.asarray(Wv, np.float32).T))
    Wo = _round_tf32(np.ascontiguousarray(np.asarray(Wo, np.float32)))
    # MT[(u,v),(p,q)] = qk[13+u-p, 13+v-q]  (pure gather, no arithmetic)
    qk2 = np.asarray(qk, np.float32).reshape(KS, KS)
    idx = (KS // 2) + np.arange(IMG)[:, None] - np.arange(IMG)[None, :]
    MT = _round_tf32(np.ascontiguousarray(
        qk2[idx[:, None, :, None], idx[None, :, None, :]].reshape(NPOS, NPOS)
    ))
    bias = np.ascontiguousarray(
        np.broadcast_to(np.asarray(bo, np.float32), (128, DIM))
    )
    return XTC, WvT, Wo, MT, bias


def _run(x, Wv, qk, Wo, bo, **spmd_kwargs):
    XTC, WvT, Wo_, MT, bias = _host_prep(x, Wv, qk, Wo, bo)
    nc = _get_program()
    in_maps = [
        {"xt": XTC[c], "wvt": WvT, "wo": Wo_, "mt": MT, "bias": bias}
        for c in range(N_CORES)
    ]
    res = run_bass_kernel_spmd(nc, in_maps, list(range(N_CORES)), **spmd_kwargs)
    y = np.concatenate([res.results[c]["y"] for c in range(N_CORES)], axis=0)
    return y, res


def kernel(x, Wv, qk, Wo, bo):
    y, _ = _run(x, Wv, qk, Wo, bo)
    return y


# revision 9
# speedup vs baseline: 1.1177x; 1.0367x over previous
# Trainium2 Bass kernel for nn_Attention_88313117540497.
#
# Reference computation (per batch b of 128):
#   v = x_b @ Wv                      (196, 384) @ (384, 512)
#   conv: each of the 512 channels' 14x14 image convolved with a 27x27
#         kernel qk at padding 13 -> same 14x14 output
#   y = conv_out @ Wo + bo            (196, 512) @ (512, 384)
#
# Key observations:
#  1. A 27x27 kernel on a 14x14 image with padding 13 covers every input
#     pixel for every output pixel, so the conv is exactly a dense linear
#     map over the 196 positions: out[p] = sum_u M[p, u] * img[u] with
#     M[(p,q),(u,v)] = qk[13+u-p, 13+v-q]. M is shared across all
#     batches and channels, so conv == matmul with a 196x196 matrix.
#  2. The whole module is then  y_b = M @ x_b @ Wv @ Wo + bo.  Folding
#     W = Wv @ Wo (384x384, computed once on device) removes the
#     INNER=512 dimension entirely: y_b = M @ (x_b @ W) + bo, which
#     halves the FLOPs.
#  3. Feeding x transposed (features major) makes both matmuls natural
#     for the PE (contraction dim on partitions for both operands, no
#     on-device transposes):
#        H_b = XT_b.T @ W      (lhsT = XT_b, rhs = W)   -> (196, 384)
#        Y_b = MT.T @ H_b      (lhsT = MT,   rhs = H_b) -> (196, 384)
#     with MT = M^T. All matmuls have free dim N = 384 >= 256, so
#     float32r runs at 1 cycle/row (4x faster than plain fp32).
#
# Sharding: data-parallel over batch, 16 batches per core, weights
# replicated. No collectives needed.
#
# DMA layout (from trace analysis): each dma_start lands on one DMA
# queue; reads get 12 queues, writes only 4, and throughput per queue is
# dominated by per-partition contiguous run length. So: x is host-packed
# feature-major (12.5KB DRAM rows) and loaded as 24 medium DMAs with 6
# in-flight buffers; y is written in a PE-native k-major grouped layout
# (6KB runs) and reassembled on the host; small constants go through the
# otherwise-idle SWDGE.

import numpy as np

import concourse.bass as bass
from concourse import bacc
import concourse.mybir as mybir
import concourse.tile as tile
from concourse.bass_utils import run_bass_kernel_spmd

N_CORES = 8
B = 128                 # total batch
BPC = B // N_CORES      # batches per core
DIM = 384
INNER = 512
NPOS = 196              # 14*14 positions
IMG = 14
KS = 27                 # conv kernel size

F32 = mybir.dt.float32
F32R = mybir.dt.float32r

TOK_CHUNKS = [(0, 128), (128, 68)]
DCH = DIM // 128        # 3 feature chunks (contraction of stage 1)
ICH = INNER // 128      # 4 inner chunks (contraction of the fold)
GX = 2                  # batches per X-load group
NGX = BPC // GX
GY = 4                  # batches per Y-store group
NGY = BPC // GY

# float32r (= tfloat32) runs at full PE rate for free dim >= 256. The
# BIR verifier requires producers of f32r matmul operands to write
# pre-rounded TF32: DMA-fed operands are rounded on the host, on-chip
# producers (PSUM evictions) write float32r directly.
MM_DT = F32R


def build_program():
    nc = bacc.Bacc("TRN2", debug=False)

    # x, feature-major: [feature chunk, partition (feature%128), token]
    xt_d = nc.dram_tensor("xt", [DCH, 128, BPC * NPOS], MM_DT,
                          kind="ExternalInput")
    wvt_d = nc.dram_tensor("wvt", [INNER, DIM], MM_DT, kind="ExternalInput")
    wo_d = nc.dram_tensor("wo", [INNER, DIM], MM_DT, kind="ExternalInput")
    mt_d = nc.dram_tensor("mt", [NPOS, NPOS], MM_DT, kind="ExternalInput")
    bias_d = nc.dram_tensor("bias", [128, DIM], MM_DT, kind="ExternalInput")
    # y, PE-native: [group, p-chunk k, partition, batch-in-group, e]
    y_d = nc.dram_tensor("y", [NGY, 2, 128, GY, DIM], F32,
                         kind="ExternalOutput")

    GT = GX * NPOS       # tokens per X group

    with tile.TileContext(nc) as tc:
        with (
            tc.tile_pool(name="const", bufs=1) as const,
            tc.tile_pool(name="work", bufs=2) as work,
            tc.tile_pool(name="psum", bufs=2, space="PSUM") as psum,
        ):
            dges = [nc.sync, nc.scalar]

            # ---- small constants via SWDGE (keeps HWDGE queues free) ----
            bias_sb = const.tile([128, DIM], MM_DT)
            nc.gpsimd.dma_start(bias_sb[:, :], bias_d[:, :])
            mt_sb = const.tile([128, 2 * NPOS], MM_DT)
            for uc, (u0, usz) in enumerate(TOK_CHUNKS):
                nc.gpsimd.dma_start(
                    mt_sb[:usz, uc * NPOS:(uc + 1) * NPOS],
                    mt_d[u0:u0 + usz, :],
                )

            # ---- weights over both HWDGE trigger engines ----
            wvt_sb = const.tile([128, ICH * DIM], MM_DT)
            wo_sb = const.tile([128, ICH * DIM], MM_DT)
            for c in range(ICH):
                dges[c % 2].dma_start(
                    wvt_sb[:, c * DIM:(c + 1) * DIM],
                    wvt_d[c * 128:(c + 1) * 128, :],
                )
                dges[(c + 1) % 2].dma_start(
                    wo_sb[:, c * DIM:(c + 1) * DIM],
                    wo_d[c * 128:(c + 1) * 128, :],
                )

            # ---- PE warm-up gated on the small bias load, so the clock
            # is ramped when the fold + stage-1 stream begins ----
            for wi in range(10):
                warm = psum.tile([128, DIM], F32, tag="y1", name=f"warm{wi}")
                nc.tensor.matmul(
                    warm[:, :],
                    lhsT=bias_sb[:, 0:128],
                    rhs=bias_sb[:, 0:DIM],
                    start=True,
                    stop=True,
                )

            # ---- stage 0: fold W = Wv @ Wo  (W[d, e], d on partitions) ----
            w_sb = const.tile([128, DCH * DIM], MM_DT)
            fold_tags = ["h0", "h1", "y0"]
            for di in range(DCH):
                pw = psum.tile([128, DIM], F32, tag=fold_tags[di], name=f"pw{di}")
                for c in range(ICH):
                    nc.tensor.matmul(
                        pw[:, :],
                        lhsT=wvt_sb[:, c * DIM + di * 128:c * DIM + (di + 1) * 128],
                        rhs=wo_sb[:, c * DIM:(c + 1) * DIM],
                        start=(c == 0),
                        stop=(c == ICH - 1),
                    )
                nc.scalar.copy(w_sb[:, di * DIM:(di + 1) * DIM], pw[:, :])

            # ---- main loop ----
            xt_t = None
            y_t = None
            for b in range(BPC):
                if b % GX == 0:
                    g = b // GX
                    xt_t = work.tile([128, DCH * GT], MM_DT, tag="xt", bufs=6,
                                     name=f"xt{g}")
                    # one DMA per feature chunk -> 3 queues in parallel
                    for c in range(DCH):
                        dges[(g + c) % 2].dma_start(
                            xt_t[:, c * GT:(c + 1) * GT],
                            xt_d[c, :, g * GT:(g + 1) * GT],
                        )
                if b % GY == 0:
                    # [k-chunk, batch-in-group, e] per partition
                    y_t = work.tile([128, 2 * GY * DIM], F32, tag="y",
                                    name=f"y{b // GY}")

                tok0 = (b % GX) * NPOS
                bi = b % GY

                # stage 1: H_b = XT_b.T @ W  (tokens on partitions)
                h_t = work.tile([128, 2 * DIM], MM_DT, tag="h", bufs=3,
                                name=f"h{b}")
                for t, (u0, usz) in enumerate(TOK_CHUNKS):
                    ph = psum.tile([128, DIM], F32, tag=f"h{t}", name=f"ph{t}_{b}")
                    for c in range(DCH):
                        o = c * GT + tok0 + u0
                        nc.tensor.matmul(
                            ph[:usz, :],
                            lhsT=xt_t[:, o:o + usz],
                            rhs=w_sb[:, c * DIM:(c + 1) * DIM],
                            start=(c == 0),
                            stop=(c == DCH - 1),
                        )
                    nc.scalar.copy(h_t[:usz, t * DIM:(t + 1) * DIM], ph[:usz, :])

                # stage 2: Y_b = MT.T @ H_b + bias
                for t2, (p0, psz) in enumerate(TOK_CHUNKS):
                    py = psum.tile([128, DIM], F32, tag=f"y{t2}", name=f"py{t2}_{b}")
                    for uc, (u0, usz) in enumerate(TOK_CHUNKS):
                        nc.tensor.matmul(
                            py[:psz, :],
                            lhsT=mt_sb[:usz, uc * NPOS + p0:uc * NPOS + p0 + psz],
                            rhs=h_t[:usz, uc * DIM:(uc + 1) * DIM],
                            start=(uc == 0),
                            stop=(uc == 1),
                        )
                    nc.vector.tensor_add(
                        y_t[:psz, t2 * GY * DIM + bi * DIM:
                            t2 * GY * DIM + (bi + 1) * DIM],
                        py[:psz, :],
                        bias_sb[:psz, :].bitcast(F32),
                    )

                if b % GY == GY - 1:
                    g = b // GY
                    # k-major output: per partition runs of GY*384 f32 (6KB)
                    dges[g % 2].dma_start(
                        y_d[g, 0], y_t[:, 0:GY * DIM])
                    dges[(g + 1) % 2].dma_start(
                        y_d[g, 1, 0:68], y_t[:68, GY * DIM:2 * GY * DIM])

    nc.compile()
    return nc


_PROGRAM = None


def _get_program():
    global _PROGRAM
    if _PROGRAM is None:
        _PROGRAM = build_program()
    return _PROGRAM


def _round_tf32(a):
    # round-to-nearest to the 10-bit TF32 mantissa (dtype-format conversion
    # for the float32r DRAM tensors)
    b = (a.view(np.uint32) + np.uint32(0x1000)) & np.uint32(0xFFFFE000)
    return b.view(np.float32)


def _host_prep(x, Wv, qk, Wo, bo):
    x = np.asarray(x, dtype=np.float32)
    # per-core feature-major token stream: (cores, 3, 128, BPC*196)
    XTC = np.ascontiguousarray(
        x.reshape(N_CORES, BPC * NPOS, DIM).transpose(0, 2, 1)
    ).reshape(N_CORES, DCH, 128, BPC * NPOS)
    XTC = _round_tf32(XTC)
    WvT = _round_tf32(np.ascontiguousarray(np.asarray(Wv, np.float32).T))
    Wo = _round_tf32(np.ascontiguousarray(np.asarray(Wo, np.float32)))
    # MT[(u,v),(p,q)] = qk[13+u-p, 13+v-q]  (pure gather, no arithmetic)
    qk2 = np.asarray(qk, np.float32).reshape(KS, KS)
    idx = (KS // 2) + np.arange(IMG)[:, None] - np.arange(IMG)[None, :]
    MT = _round_tf32(np.ascontiguousarray(
        qk2[idx[:, None, :, None], idx[None, :, None, :]].reshape(NPOS, NPOS)
    ))
    bias = np.ascontiguousarray(
        np.broadcast_to(np.asarray(bo, np.float32), (128, DIM))
    )
    return XTC, WvT, Wo, MT, bias


def _unpack_core(y2):
    # y2: [NGY, 2, 128, GY, DIM] -> (BPC, NPOS, DIM)
    out = np.empty((BPC, NPOS, DIM), np.float32)
    top = y2[:, 0].transpose(0, 2, 1, 3)          # [NGY, GY, 128, DIM]
    bot = y2[:, 1, 0:68].transpose(0, 2, 1, 3)    # [NGY, GY, 68, DIM]
    out[:, 0:128, :] = top.reshape(BPC, 128, DIM)
    out[:, 128:NPOS, :] = bot.reshape(BPC, 68, DIM)
    return out


def _run(x, Wv, qk, Wo, bo, **spmd_kwargs):
    XTC, WvT, Wo_, MT, bias = _host_prep(x, Wv, qk, Wo, bo)
    nc = _get_program()
    in_maps = [
        {"xt": XTC[c], "wvt": WvT, "wo": Wo_, "mt": MT, "bias": bias}
        for c in range(N_CORES)
    ]
    res = run_bass_kernel_spmd(nc, in_maps, list(range(N_CORES)), **spmd_kwargs)
    y = np.concatenate(
        [_unpack_core(res.results[c]["y"]) for c in range(N_CORES)], axis=0)
    return y, res


def kernel(x, Wv, qk, Wo, bo):
    y, _ = _run(x, Wv, qk, Wo, bo)
    return y


# revision 11
# speedup vs baseline: 1.1366x; 1.0169x over previous
# Trainium2 Bass kernel for nn_Attention_88313117540497.
#
# Reference computation (per batch b of 128):
#   v = x_b @ Wv                      (196, 384) @ (384, 512)
#   conv: each of the 512 channels' 14x14 image convolved with a 27x27
#         kernel qk at padding 13 -> same 14x14 output
#   y = conv_out @ Wo + bo            (196, 512) @ (512, 384)
#
# Key observations:
#  1. A 27x27 kernel on a 14x14 image with padding 13 covers every input
#     pixel for every output pixel, so the conv is exactly a dense linear
#     map over the 196 positions: out[p] = sum_u M[p, u] * img[u] with
#     M[(p,q),(u,v)] = qk[13+u-p, 13+v-q]. M is shared across all
#     batches and channels, so conv == matmul with a 196x196 matrix.
#  2. The whole module is then  y_b = M @ x_b @ Wv @ Wo + bo.  Folding
#     W = Wv @ Wo (384x384, computed once on device) removes the
#     INNER=512 dimension entirely: y_b = M @ (x_b @ W) + bo, which
#     halves the FLOPs.
#  3. Feeding x transposed (features major) makes both matmuls natural
#     for the PE (contraction dim on partitions for both operands, no
#     on-device transposes):
#        H_b = XT_b.T @ W      (lhsT = XT_b, rhs = W)   -> (196, 384)
#        Y_b = MT.T @ H_b      (lhsT = MT,   rhs = H_b) -> (196, 384)
#     with MT = M^T. All matmuls have free dim N = 384 >= 256, so
#     float32r runs at 1 cycle/row (4x faster than plain fp32).
#
# Sharding: data-parallel over batch, 16 batches per core, weights
# replicated. No collectives needed.
#
# DMA layout (from trace analysis): each dma_start lands on one DMA
# queue; reads get 12 queues, writes only 4, and throughput per queue is
# dominated by per-partition contiguous run length. So: x is host-packed
# feature-major (12.5KB DRAM rows) and loaded as 24 medium DMAs with 6
# in-flight buffers; y is written in a PE-native k-major grouped layout
# (6KB runs) and reassembled on the host; small constants go through the
# otherwise-idle SWDGE.

import numpy as np

import concourse.bass as bass
from concourse import bacc
import concourse.mybir as mybir
import concourse.tile as tile
from concourse.bass_utils import run_bass_kernel_spmd

N_CORES = 8
B = 128                 # total batch
BPC = B // N_CORES      # batches per core
DIM = 384
INNER = 512
NPOS = 196              # 14*14 positions
IMG = 14
KS = 27                 # conv kernel size

F32 = mybir.dt.float32
F32R = mybir.dt.float32r

TOK_CHUNKS = [(0, 128), (128, 68)]
DCH = DIM // 128        # 3 feature chunks (contraction of stage 1)
ICH = INNER // 128      # 4 inner chunks (contraction of the fold)
GX = 2                  # batches per X-load group
NGX = BPC // GX
GY = 4                  # batches per Y-store group
NGY = BPC // GY

# float32r (= tfloat32) runs at full PE rate for free dim >= 256. The
# BIR verifier requires producers of f32r matmul operands to write
# pre-rounded TF32: DMA-fed operands are rounded on the host, on-chip
# producers (PSUM evictions) write float32r directly.
MM_DT = F32R


def build_program():
    nc = bacc.Bacc("TRN2", debug=False)

    # x, feature-major: [feature chunk, partition (feature%128), token]
    xt_d = nc.dram_tensor("xt", [DCH, 128, BPC * NPOS], MM_DT,
                          kind="ExternalInput")
    wvt_d = nc.dram_tensor("wvt", [INNER, DIM], MM_DT, kind="ExternalInput")
    wo_d = nc.dram_tensor("wo", [INNER, DIM], MM_DT, kind="ExternalInput")
    mt_d = nc.dram_tensor("mt", [NPOS, NPOS], MM_DT, kind="ExternalInput")
    bias_d = nc.dram_tensor("bias", [128, DIM], MM_DT, kind="ExternalInput")
    # y, PE-native: [group, p-chunk k, partition, batch-in-group, e]
    y_d = nc.dram_tensor("y", [NGY, 2, 128, GY, DIM], F32,
                         kind="ExternalOutput")

    GT = GX * NPOS       # tokens per X group

    with tile.TileContext(nc) as tc:
        with (
            tc.tile_pool(name="const", bufs=1) as const,
            tc.tile_pool(name="work", bufs=2) as work,
            tc.tile_pool(name="psum", bufs=2, space="PSUM") as psum,
        ):
            dges = [nc.sync, nc.scalar]

            # ---- small constants via SWDGE (keeps HWDGE queues free) ----
            bias_sb = const.tile([128, DIM], MM_DT)
            nc.gpsimd.dma_start(bias_sb[:, :], bias_d[:, :])
            mt_sb = const.tile([128, 2 * NPOS], MM_DT)
            for uc, (u0, usz) in enumerate(TOK_CHUNKS):
                nc.gpsimd.dma_start(
                    mt_sb[:usz, uc * NPOS:(uc + 1) * NPOS],
                    mt_d[u0:u0 + usz, :],
                )

            # ---- weights over both HWDGE trigger engines ----
            wvt_sb = const.tile([128, ICH * DIM], MM_DT)
            wo_sb = const.tile([128, ICH * DIM], MM_DT)
            for c in range(ICH):
                dges[c % 2].dma_start(
                    wvt_sb[:, c * DIM:(c + 1) * DIM],
                    wvt_d[c * 128:(c + 1) * 128, :],
                )
                dges[(c + 1) % 2].dma_start(
                    wo_sb[:, c * DIM:(c + 1) * DIM],
                    wo_d[c * 128:(c + 1) * 128, :],
                )

            # ---- PE warm-up on a memset tile (no DMA dependency): keeps
            # the tensor engine busy from t~0 so the clock is ramped when
            # the fold + stage-1 stream begins ----
            warm_f32 = const.tile([128, DIM], F32)
            nc.gpsimd.memset(warm_f32[:, :], 0.0)
            warm_src = const.tile([128, DIM], MM_DT)
            nc.vector.tensor_copy(warm_src[:, :], warm_f32[:, :])
            for wi in range(24):
                warm = psum.tile([128, DIM], F32, tag="y1", name=f"warm{wi}")
                nc.tensor.matmul(
                    warm[:, :],
                    lhsT=warm_src[:, 0:128],
                    rhs=warm_src[:, 0:DIM],
                    start=True,
                    stop=True,
                )

            # ---- stage 0: fold W = Wv @ Wo  (W[d, e], d on partitions) ----
            w_sb = const.tile([128, DCH * DIM], MM_DT)
            fold_tags = ["h0", "h1", "y0"]
            for di in range(DCH):
                pw = psum.tile([128, DIM], F32, tag=fold_tags[di], name=f"pw{di}")
                for c in range(ICH):
                    nc.tensor.matmul(
                        pw[:, :],
                        lhsT=wvt_sb[:, c * DIM + di * 128:c * DIM + (di + 1) * 128],
                        rhs=wo_sb[:, c * DIM:(c + 1) * DIM],
                        start=(c == 0),
                        stop=(c == ICH - 1),
                    )
                nc.scalar.copy(w_sb[:, di * DIM:(di + 1) * DIM], pw[:, :])

            # ---- main loop ----
            xt_t = None
            y_t = None
            for b in range(BPC):
                if b % GX == 0:
                    g = b // GX
                    xt_t = work.tile([128, DCH * GT], MM_DT, tag="xt", bufs=6,
                                     name=f"xt{g}")
                    # one DMA per feature chunk -> 3 queues in parallel
                    for c in range(DCH):
                        nc.sync.dma_start(
                            xt_t[:, c * GT:(c + 1) * GT],
                            xt_d[c, :, g * GT:(g + 1) * GT],
                        )
                if b % GY == 0:
                    # [k-chunk, batch-in-group, e] per partition
                    y_t = work.tile([128, 2 * GY * DIM], F32, tag="y",
                                    name=f"y{b // GY}")

                tok0 = (b % GX) * NPOS
                bi = b % GY

                # stage 1: H_b = XT_b.T @ W  (tokens on partitions)
                h_t = work.tile([128, 2 * DIM], MM_DT, tag="h", bufs=3,
                                name=f"h{b}")
                for t, (u0, usz) in enumerate(TOK_CHUNKS):
                    ph = psum.tile([128, DIM], F32, tag=f"h{t}", name=f"ph{t}_{b}")
                    for c in range(DCH):
                        o = c * GT + tok0 + u0
                        nc.tensor.matmul(
                            ph[:usz, :],
                            lhsT=xt_t[:, o:o + usz],
                            rhs=w_sb[:, c * DIM:(c + 1) * DIM],
                            start=(c == 0),
                            stop=(c == DCH - 1),
                        )
                    nc.scalar.copy(h_t[:usz, t * DIM:(t + 1) * DIM], ph[:usz, :])

                # stage 2: Y_b = MT.T @ H_b + bias
                for t2, (p0, psz) in enumerate(TOK_CHUNKS):
                    py = psum.tile([128, DIM], F32, tag=f"y{t2}", name=f"py{t2}_{b}")
                    for uc, (u0, usz) in enumerate(TOK_CHUNKS):
                        nc.tensor.matmul(
                            py[:psz, :],
                            lhsT=mt_sb[:usz, uc * NPOS + p0:uc * NPOS + p0 + psz],
                            rhs=h_t[:usz, uc * DIM:(uc + 1) * DIM],
                            start=(uc == 0),
                            stop=(uc == 1),
                        )
                    nc.vector.tensor_add(
                        y_t[:psz, t2 * GY * DIM + bi * DIM:
                            t2 * GY * DIM + (bi + 1) * DIM],
                        py[:psz, :],
                        bias_sb[:psz, :].bitcast(F32),
                    )

                if b % GY == GY - 1:
                    g = b // GY
                    # k-major output: per partition runs of GY*384 f32 (6KB)
                    dges[g % 2].dma_start(
                        y_d[g, 0], y_t[:, 0:GY * DIM])
                    nc.gpsimd.dma_start(
                        y_d[g, 1, 0:68], y_t[:68, GY * DIM:2 * GY * DIM])

    nc.compile()
    return nc


_PROGRAM = None


def _get_program():
    global _PROGRAM
    if _PROGRAM is None:
        _PROGRAM = build_program()
    return _PROGRAM


def _round_tf32(a):
    # round-to-nearest to the 10-bit TF32 mantissa (dtype-format conversion
    # for the float32r DRAM tensors)
    b = (a.view(np.uint32) + np.uint32(0x1000)) & np.uint32(0xFFFFE000)
    return b.view(np.float32)


def _host_prep(x, Wv, qk, Wo, bo):
    x = np.asarray(x, dtype=np.float32)
    # per-core feature-major token stream: (cores, 3, 128, BPC*196)
    XTC = np.ascontiguousarray(
        x.reshape(N_CORES, BPC * NPOS, DIM).transpose(0, 2, 1)
    ).reshape(N_CORES, DCH, 128, BPC * NPOS)
    XTC = _round_tf32(XTC)
    WvT = _round_tf32(np.ascontiguousarray(np.asarray(Wv, np.float32).T))
    Wo = _round_tf32(np.ascontiguousarray(np.asarray(Wo, np.float32)))
    # MT[(u,v),(p,q)] = qk[13+u-p, 13+v-q]  (pure gather, no arithmetic)
    qk2 = np.asarray(qk, np.float32).reshape(KS, KS)
    idx = (KS // 2) + np.arange(IMG)[:, None] - np.arange(IMG)[None, :]
    MT = _round_tf32(np.ascontiguousarray(
        qk2[idx[:, None, :, None], idx[None, :, None, :]].reshape(NPOS, NPOS)
    ))
    bias = np.ascontiguousarray(
        np.broadcast_to(np.asarray(bo, np.float32), (128, DIM))
    )
    return XTC, WvT, Wo, MT, bias


def _unpack_core(y2):
    # y2: [NGY, 2, 128, GY, DIM] -> (BPC, NPOS, DIM)
    out = np.empty((BPC, NPOS, DIM), np.float32)
    top = y2[:, 0].transpose(0, 2, 1, 3)          # [NGY, GY, 128, DIM]
    bot = y2[:, 1, 0:68].transpose(0, 2, 1, 3)    # [NGY, GY, 68, DIM]
    out[:, 0:128, :] = top.reshape(BPC, 128, DIM)
    out[:, 128:NPOS, :] = bot.reshape(BPC, 68, DIM)
    return out


def _run(x, Wv, qk, Wo, bo, **spmd_kwargs):
    XTC, WvT, Wo_, MT, bias = _host_prep(x, Wv, qk, Wo, bo)
    nc = _get_program()
    in_maps = [
        {"xt": XTC[c], "wvt": WvT, "wo": Wo_, "mt": MT, "bias": bias}
        for c in range(N_CORES)
    ]
    res = run_bass_kernel_spmd(nc, in_maps, list(range(N_CORES)), **spmd_kwargs)
    y = np.concatenate(
        [_unpack_core(res.results[c]["y"]) for c in range(N_CORES)], axis=0)
    return y, res


def kernel(x, Wv, qk, Wo, bo):
    y, _ = _run(x, Wv, qk, Wo, bo)
    return y
